# revision 29
# baseline (speedup 1.0000x reference)
"""Multi-head attention (B=2, S=2048, D=1024, H=16) on 8 Trainium2 NeuronCores.

Sharding: data-parallel over batch (cores 0-3 -> b=0, cores 4-7 -> b=1) x
tensor-parallel over heads (4 heads per core, column-parallel QKV, row-parallel
output projection; partial outputs summed on the host).

Per-core dataflow (all matmul contraction dims live on SBUF partitions; all
matmul operands fp16, fp32 PSUM accumulation):
  XT_{Q,K,V} [D, S] (host pre-transposed fp16) --proj--> QT/KT [dkpair, q],
  V [k, dk] --scoresT [k, q] (row-packed head pairs, 2-bank PSUM spans,
  double-buffered)--> exp fp16 (one ACT instr per span: PSUM->SBUF with
  scale=1/sqrt(dk) and the mask folded in as a per-k bias) --> ctx [dkpair, q]
  (col-packed pairs, single PSUM accumulation group opened/closed by K=1
  zero-matmuls) ; softmax denominator = fp16 pairwise tree (DVE) + ones-matmul
  finisher (PE) -> fast reciprocal -> DRAM round-trip broadcast ; output
  projection fp16, partial outputs summed on the host.
  Tile's per-engine schedules are static, so projection/output work is emitted
  in small units inside the attention k-loop (earliest-deadline-first) to fill
  PE stalls; bulk input DMA rides the sync HWDGE queue while stores and
  broadcasts use the gpsimd SWDGE queue.
"""

import os
import sys

for _p in ("/opt/trn_rl_repo", "/root/.axon_site/_ro/trn_rl_repo"):
    if os.path.isdir(_p) and _p not in sys.path:
        sys.path.insert(0, _p)

import numpy as np

import concourse.bass as bass
import concourse.mybir as mybir
import concourse.tile as tile
from concourse import bacc
from concourse.bass_utils import run_bass_kernel_spmd

F32 = mybir.dt.float32
F32R = mybir.dt.float32r
F16 = mybir.dt.float16
AFT = mybir.ActivationFunctionType

# Full-problem constants
B, S, D, H = 2, 2048, 1024, 16
DK = D // H  # 64
HL = 4  # heads per core
NCORES = 8


def build_core(s=S, d=D, hl=HL, qb=512, dt_qk=F16, dt_att=F16, dt_out=F16):
    """Build the per-core Bass program (same program on every core; per-core
    tensors differ only in contents)."""
    assert hl % 2 == 0
    npair = hl // 2
    nkc = s // 128  # k-chunks
    ndc = d // 128  # d-chunks (QKV contraction)
    nqb = s // qb  # q blocks
    njp = nkc // 2  # V-projection chunk pairs
    span = hl * qb  # free size of one scores/exp span
    assert qb <= 512 and span * 4 <= 16 * 1024  # fits alongside other psum
    nout = d // 512  # output projection N-halves

    nc = bacc.Bacc("TRN2", target_bir_lowering=False)

    # ---- DRAM parameters -------------------------------------------------
    xtq = nc.declare_dram_parameter("XTQ", [d, s], dt_qk, isOutput=False)
    xtk = nc.declare_dram_parameter("XTK", [d, s], dt_qk, isOutput=False)
    xtv = nc.declare_dram_parameter("XTV", [d, s], dt_qk, isOutput=False)
    wqt = nc.declare_dram_parameter("WQT", [d, hl * DK], dt_qk, isOutput=False)
    wkt = nc.declare_dram_parameter("WKT", [d, hl * DK], dt_qk, isOutput=False)
    wvt = nc.declare_dram_parameter("WVT", [d, hl * DK], dt_qk, isOutput=False)
    wot = nc.declare_dram_parameter("WOT", [hl * DK, d], dt_out, isOutput=False)
    bq = nc.declare_dram_parameter("BQ", [hl * DK], F32, isOutput=False)
    bk = nc.declare_dram_parameter("BK", [hl * DK], F32, isOutput=False)
    bv = nc.declare_dram_parameter("BV", [hl * DK], dt_qk, isOutput=False)
    moff = nc.declare_dram_parameter("MOFF", [s], F32, isOutput=False)
    onesr = nc.declare_dram_parameter("ONESR", [128], dt_qk, isOutput=False)
    rscratch = nc.dram_tensor("rscratch", [s // qb * 4, qb], F32)
    out = nc.declare_dram_parameter("OUT", [s, d], F32, isOutput=True)

    with tile.TileContext(nc) as tc:
        with (
            tc.tile_pool(name="res", bufs=1) as res,
            tc.tile_pool(name="xt", bufs=6) as xt_pool,
            tc.tile_pool(name="xv", bufs=3) as xv_pool,
            tc.tile_pool(name="exp", bufs=6) as exp_pool,
            tc.tile_pool(name="tree", bufs=12) as tree_pool,
            tc.tile_pool(name="small", bufs=8) as small_pool,
            tc.tile_pool(name="ostage", bufs=3) as ostage_pool,
            tc.tile_pool(name="scores", bufs=2, space="PSUM") as scores_pool,
            tc.tile_pool(name="ctxp", bufs=2, space="PSUM") as ctx_pool,
            tc.tile_pool(name="mm", bufs=2, space="PSUM") as mm_pool,
        ):
            # ---- resident tiles + preamble DMAs --------------------------
            qt_sb = res.tile([128, npair, s], dt_qk, tag="qt")
            kt_sb = res.tile([128, npair, s], dt_qk, tag="kt")
            v_sb = res.tile([128, nkc, npair, 128], dt_att, tag="v")
            cct_sb = res.tile([128, npair, s], dt_out, tag="cct")
            wqt_sb = res.tile([128, ndc, hl * DK], dt_qk, tag="wqt")
            wkt_sb = res.tile([128, ndc, hl * DK], dt_qk, tag="wkt")
            wvt_sb = res.tile([128, ndc, hl * DK], dt_qk, tag="wvt")
            wot_sb = res.tile([128, npair, d], dt_out, tag="wot")
            bq_sb = res.tile([128, npair], F32, tag="bq")
            bk_sb = res.tile([128, npair], F32, tag="bk")
            bv_sb = res.tile([1, hl * DK], dt_qk, tag="bv")
            moff_sb = res.tile([128, nkc], F32, tag="moff")
            ones_r = res.tile([1, 128], dt_qk, tag="ones_r")
            ones_a = res.tile([128, 1], dt_att, tag="ones_a")
            z128 = res.tile([1, 128], dt_att, tag="z128")
            zq = res.tile([1, 512], dt_att, tag="zq")

            nc.sync.dma_start(out=wqt_sb, in_=wqt[:].rearrange("(c p) n -> p c n", p=128))
            nc.sync.dma_start(out=wkt_sb, in_=wkt[:].rearrange("(c p) n -> p c n", p=128))
            nc.sync.dma_start(out=bq_sb, in_=bq[:].rearrange("(c p) -> p c", p=128))
            nc.sync.dma_start(out=bk_sb, in_=bk[:].rearrange("(c p) -> p c", p=128))
            nc.sync.dma_start(out=moff_sb, in_=moff[:].rearrange("(c p) -> p c", p=128))
            nc.sync.dma_start(out=ones_r, in_=onesr[:].rearrange("(o n) -> o n", o=1))
            nc.vector.memset(ones_a, 1.0)
            nc.vector.memset(z128, 0.0)
            nc.vector.memset(zq, 0.0)

            # ---- phase helpers -------------------------------------------
            def qk_proj(qc, xdram, w_sb, b_sb, o_sb):
                """One 512-wide block of the Q/K projection (both head pairs
                share each streamed XT tile)."""
                sl = slice(qc * 512, (qc + 1) * 512)
                psums = [
                    mm_pool.tile([128, 512], F32, tag="mm", name=f"qkp{i}")
                    for i in range(npair)
                ]
                for dc in range(ndc):
                    xt_t = xt_pool.tile([128, 512], dt_qk, tag="xt")
                    nc.sync.dma_start(out=xt_t, in_=xdram[:][dc * 128 : (dc + 1) * 128, sl])
                    for pr in range(npair):
                        nc.tensor.matmul(
                            psums[pr],
                            lhsT=w_sb[:, dc, pr * 128 : (pr + 1) * 128],
                            rhs=xt_t,
                            start=(dc == 0),
                            stop=(dc == ndc - 1),
                        )
                for pr in range(npair):
                    nc.vector.tensor_scalar_add(
                        o_sb[:, pr, sl], psums[pr], b_sb[:, pr : pr + 1]
                    )

            def v_proj(jp):
                """V projection for k-chunks 2*jp, 2*jp+1 -> v_sb fp16."""
                psums = [mm_pool.tile([128, hl * DK], F32, tag="mm", name=f"vp{i}") for i in range(2)]
                for dc in range(ndc):
                    xv_t = xv_pool.tile([128, 256], dt_qk, tag="xv")
                    nc.sync.dma_start(
                        out=xv_t,
                        in_=xtv[:][dc * 128 : (dc + 1) * 128, jp * 256 : (jp + 1) * 256],
                    )
                    for m in range(2):
                        nc.tensor.matmul(
                            psums[m],
                            lhsT=xv_t[:, m * 128 : (m + 1) * 128],
                            rhs=wvt_sb[:, dc, :],
                            start=(dc == 0),
                            stop=False,
                        )
                for m in range(2):
                    # bias: += ones[k] * bv[dk]
                    nc.tensor.matmul(psums[m], lhsT=ones_r, rhs=bv_sb, start=False, stop=True)
                    j = 2 * jp + m
                    for pr in range(npair):
                        nc.vector.tensor_copy(
                            v_sb[:, j, pr, :], psums[m][:, pr * 128 : (pr + 1) * 128]
                        )

            def attention_qb(qb_i, extra_emit):
                """All k-chunks of one q block: scores -> exp -> ctx (+sum tree),
                then normalize into cct_sb."""
                qsl = slice(qb_i * qb, (qb_i + 1) * qb)
                ctxp = [ctx_pool.tile([128, qb], F32, tag="ctx", name=f"ctx{i}") for i in range(npair)]
                for pr in range(npair):
                    # K=1 zero-matmul opens the bank's single accumulation
                    # group across all 128 partitions (both col-packed heads)
                    nc.tensor.matmul(
                        ctxp[pr], lhsT=z128, rhs=zq[:, :qb], start=True, stop=False
                    )
                # per-pair binary-counter pairwise fold of exp half-spans
                partials = [[] for _ in range(npair)]

                def fold(pr, t, lvl):
                    pl = partials[pr]
                    while pl and pl[-1][1] == lvl:
                        prev, _ = pl.pop()
                        nt = tree_pool.tile([128, 2 * qb], dt_att, tag="tree")
                        nc.vector.tensor_add(nt, prev, t)
                        t, lvl = nt, lvl + 1
                    pl.append((t, lvl))

                def emit_ctx(j, e_pair):
                    for pr in range(npair):
                        for m in range(2):
                            nc.tensor.matmul(
                                ctxp[pr][64 * m : 64 * (m + 1), :],
                                lhsT=v_sb[:, j, pr, 64 * m : 64 * (m + 1)],
                                rhs=e_pair[pr][:, m * qb : (m + 1) * qb],
                                start=False,
                                stop=False,
                            )
                        fold(pr, e_pair[pr], 0)

                prev = None  # (j, [e_t per pair]) — ctx lags scores by 1 chunk
                for j in range(nkc):
                    cur = []
                    for pr in range(npair):
                        s_t = scores_pool.tile(
                            [128, 2 * qb], F32, tag="scores", name=f"s{pr}"
                        )
                        for m in range(2):
                            nc.tensor.matmul(
                                s_t[:, m * qb : (m + 1) * qb],
                                lhsT=kt_sb[64 * m : 64 * (m + 1), pr, j * 128 : (j + 1) * 128],
                                rhs=qt_sb[64 * m : 64 * (m + 1), pr, qsl],
                                start=True,
                                stop=True,
                            )
                        e_t = exp_pool.tile([128, 2 * qb], dt_att, tag="exp")
                        nc.scalar.activation(
                            out=e_t,
                            in_=s_t,
                            func=AFT.Exp,
                            bias=moff_sb[:, j : j + 1],
                            scale=float(1.0 / np.sqrt(DK)),
                        )
                        cur.append(e_t)
                    if j in extra_emit:
                        extra_emit[j]()
                    if prev is not None:
                        emit_ctx(prev[0], prev[1])
                    prev = (j, cur)
                emit_ctx(prev[0], prev[1])

                for pr in range(npair):
                    # close the bank's accumulation group across all 128
                    # partitions (adds +0)
                    nc.tensor.matmul(
                        ctxp[pr], lhsT=z128, rhs=zq[:, :qb], start=False, stop=True
                    )

                # collapse any remaining partial sums (nkc not a power of 2)
                accs = []
                for pr in range(npair):
                    pl = partials[pr]
                    acc, _ = pl.pop()
                    while pl:
                        prev, _ = pl.pop()
                        nt = tree_pool.tile([128, 2 * qb], dt_att, tag="tree")
                        nc.vector.tensor_add(nt, prev, acc)
                        acc = nt
                    accs.append(acc)

                # sumexp per head (ones-matmul) -> fast reciprocal -> DRAM
                # round-trip to broadcast across partitions -> one mul per pair
                for h in range(hl):
                    fin = mm_pool.tile([128, 512], F32, tag="mm")
                    nc.tensor.matmul(
                        fin[0:1, :qb],
                        lhsT=ones_a,
                        rhs=accs[h // 2][:, (h % 2) * qb : (h % 2 + 1) * qb],
                        start=True,
                        stop=True,
                    )
                    rec = small_pool.tile([1, qb], F32, tag="rec")
                    rscr = small_pool.tile([1, qb], F32, tag="rscr")
                    nc.vector.reciprocal_approx_accurate(
                        out=rec, in_=fin[0:1, :qb], scratch=rscr
                    )
                    nc.gpsimd.dma_start(
                        out=rscratch[:][qb_i * hl + h, :].rearrange(
                            "(o n) -> o n", o=1
                        ),
                        in_=rec,
                    )
                rsl = rscratch[:][qb_i * hl : qb_i * hl + hl, :]
                for pr in range(npair):
                    rb = small_pool.tile([128, qb], F32, tag="rb")
                    rec_b = bass.AP(
                        tensor=rsl.tensor,
                        offset=rsl.offset + 2 * pr * qb,
                        ap=[[qb, 2], [0, 64], [1, qb]],
                    )
                    nc.gpsimd.dma_start(out=rb, in_=rec_b)
                    nc.vector.tensor_mul(cct_sb[:, pr, qsl], ctxp[pr], rb)

            def out_proj(qb_i):
                for qc in range(qb_i * (qb // 128), (qb_i + 1) * (qb // 128)):
                    csl = slice(qc * 128, (qc + 1) * 128)
                    for nh in range(nout):
                        po = mm_pool.tile([128, 512], F32, tag="mm")
                        for dc in range(npair):
                            nc.tensor.matmul(
                                po,
                                lhsT=cct_sb[:, dc, csl],
                                rhs=wot_sb[:, dc, nh * 512 : (nh + 1) * 512],
                                start=(dc == 0),
                                stop=(dc == npair - 1),
                            )
                        po_sb = ostage_pool.tile([128, 512], F32, tag="ostage")
                        nc.vector.tensor_copy(po_sb, po)
                        nc.sync.dma_start(
                            out=out[:][csl, nh * 512 : (nh + 1) * 512], in_=po_sb
                        )

            def out_proj_unit(qc, nh):
                csl = slice(qc * 128, (qc + 1) * 128)
                po = mm_pool.tile([128, 512], F32, tag="mm", name="po")
                for dc in range(npair):
                    nc.tensor.matmul(
                        po,
                        lhsT=cct_sb[:, dc, csl],
                        rhs=wot_sb[:, dc, nh * 512 : (nh + 1) * 512],
                        start=(dc == 0),
                        stop=(dc == npair - 1),
                    )
                po_sb = ostage_pool.tile([128, 512], F32, tag="ostage")
                nc.vector.tensor_copy(po_sb, po)
                nc.gpsimd.dma_start(
                    out=out[:][csl, nh * 512 : (nh + 1) * 512], in_=po_sb
                )

            # ---- emission order (sets scheduling priority) ----------------
            # Tile's per-engine schedule is static, so filler work (the other
            # projections, previous block's output projection) is emitted in
            # small units INSIDE the attention k-loop to fill PE stalls while
            # ACT drains the exp spans.
            nkb = s // 512  # 512-wide projection blocks
            qk_proj(0, xtk, wkt_sb, bk_sb, kt_sb)
            qk_proj(0, xtq, wqt_sb, bq_sb, qt_sb)
            nc.sync.dma_start(out=wvt_sb, in_=wvt[:].rearrange("(c p) n -> p c n", p=128))
            nc.sync.dma_start(out=bv_sb, in_=bv[:].rearrange("(o n) -> o n", o=1))
            nc.sync.dma_start(out=wot_sb, in_=wot[:].rearrange("(c p) n -> p c n", p=128))
            v_proj(0)
            v_proj(1)

            def fillers_for(qb_i):
                units = []  # (deadline_j, emit_fn): must be EMITTED at j<=deadline
                if qb_i == 0:
                    for kb in range(1, nkb):
                        units.append(
                            (
                                4 * kb - 1,  # fillers now follow scores(j)
                                lambda kb=kb: qk_proj(
                                    kb, xtk, wkt_sb, bk_sb, kt_sb
                                ),
                            )
                        )
                    for jp in range(2, njp):
                        # ctx(2*jp) is emitted inside loop iteration 2*jp+1
                        units.append((2 * jp + 1, lambda jp=jp: v_proj(jp)))
                else:
                    for qc in range(
                        (qb_i - 1) * (qb // 128), qb_i * (qb // 128)
                    ):
                        for nh in range(nout):
                            units.append(
                                (
                                    nkc - 1,
                                    lambda qc=qc, nh=nh: out_proj_unit(qc, nh),
                                )
                            )
                # next block's Q projection
                q_lo = -(-(qb_i + 1) * qb // 512)
                q_hi = -(-(qb_i + 2) * qb // 512) if qb_i + 1 < nqb else q_lo
                for qc in range(q_lo, q_hi):
                    units.append(
                        (
                            nkc - 1,
                            lambda qc=qc: qk_proj(qc, xtq, wqt_sb, bq_sb, qt_sb),
                        )
                    )
                # earliest-deadline-first, one unit per chunk starting at j=1
                units.sort(key=lambda t: t[0])
                sched = {}
                for i, (dl, u) in enumerate(units):
                    j = min(dl, 1 + i, nkc - 1)
                    sched.setdefault(j, []).append(u)
                return sched

            for qb_i in range(nqb):
                sched = fillers_for(qb_i)
                attention_qb(
                    qb_i,
                    {
                        j: (lambda us=us: [u() for u in us])
                        for j, us in sched.items()
                    },
                )
            # tail: last block's output projection
            for qc in range((nqb - 1) * (qb // 128), nqb * (qb // 128)):
                for nh in range(nout):
                    out_proj_unit(qc, nh)

    nc.compile()
    return nc


# ---------------------------------------------------------------------------
# Host-side wrapper
# ---------------------------------------------------------------------------

_NC_CACHE = {}


def _get_nc():
    if "nc" not in _NC_CACHE:
        _NC_CACHE["nc"] = build_core()
    return _NC_CACHE["nc"]


def kernel(pre_Q, pre_K, pre_V, Wq, bq, Wk, bk, Wv, bv, Wo, bo, mask):
    pre_Q = np.asarray(pre_Q, dtype=np.float32)
    pre_K = np.asarray(pre_K, dtype=np.float32)
    pre_V = np.asarray(pre_V, dtype=np.float32)
    Wq, Wk, Wv, Wo = (np.asarray(w, dtype=np.float32) for w in (Wq, Wk, Wv, Wo))
    bq, bk, bv, bo = (np.asarray(b_, dtype=np.float32) for b_ in (bq, bk, bv, bo))
    mask = np.asarray(mask)

    nc = _get_nc()

    # host-side shards
    xtq = [np.ascontiguousarray(pre_Q[b].T.astype(np.float16)) for b in range(B)]
    xtk = [np.ascontiguousarray(pre_K[b].T.astype(np.float16)) for b in range(B)]
    xtv = [np.ascontiguousarray(pre_V[b].T.astype(np.float16)) for b in range(B)]
    moffs = [
        np.where(mask[b, 0, 0, :] == 0, np.float32(-1e9), np.float32(0.0)).astype(
            np.float32
        )
        for b in range(B)
    ]

    in_maps = []
    for c in range(NCORES):
        b = c // 4
        hs = (c % 4) * HL * DK  # start column/row of this core's head slice
        sl = slice(hs, hs + HL * DK)
        in_maps.append(
            {
                "XTQ": xtq[b],
                "XTK": xtk[b],
                "XTV": xtv[b],
                "WQT": np.ascontiguousarray(Wq[sl, :].T.astype(np.float16)),
                "WKT": np.ascontiguousarray(Wk[sl, :].T.astype(np.float16)),
                "WVT": np.ascontiguousarray(Wv[sl, :].T.astype(np.float16)),
                "WOT": np.ascontiguousarray(Wo[:, sl].T.astype(np.float16)),
                "BQ": np.ascontiguousarray(bq[sl]),
                "BK": np.ascontiguousarray(bk[sl]),
                "BV": np.ascontiguousarray(bv[sl].astype(np.float16)),
                "MOFF": moffs[b],
                "ONESR": np.ones(128, dtype=np.float16),
            }
        )

    _NC_CACHE["last_in_maps"] = in_maps
    res = run_bass_kernel_spmd(nc, in_maps, list(range(NCORES)))
    outs = [res.results[c]["OUT"] for c in range(NCORES)]
    final = np.empty((B, S, D), dtype=np.float32)
    for b in range(B):
        acc = outs[4 * b].astype(np.float32).copy()
        for c in range(4 * b + 1, 4 * b + 4):
            acc += outs[c]
        final[b] = acc + bo[None, :]
    return final
